# revision 1
# baseline (speedup 1.0000x reference)
"""CrossFusionBlock Trainium2 kernel.

Dual-stream cross-attention block (B=8, C=512, HW=1024, 8 heads, FFN 2048).
Sharding: data-parallel over batch across 8 NeuronCores (1 batch element per
core), weights replicated. All weight transposes / bf16 casts are done on the
host so the device kernel contains no transposes at all.

Per-core dataflow (channels-first activations, [C->4x128 partitions, HW]):
  Q_cf = Wq @ X_q        (lhsT = host-supplied Wq^T, rhs = X bf16)
  K_cf = Wk @ X_kv
  V_tok = X_kv^T @ Wv^T  (token-major, lhsT = X bf16) + ones column
  S^T[tk,tq] = K_cf_head^T-slice x Q_cf_head   (K=64, auto row-tiled pairs)
  P^T = exp(S^T/8)  (no max subtraction: logits are O(1) by construction)
  AV: psum[0:64] = O_cf_head, psum[64] = Z (softmax denominator, ones column)
  O /= Z  (GPSIMD partition-broadcast of 1/Z)
  enh = Wo @ O + bo  (per-head K=64 contraction, head-major Wo^T from host)
  LN over channels via PE ones-matmul stats + GPSIMD row broadcast
  FFN: W2 @ gelu(W1 @ s + b1) + b2, residual, LN2.
"""

import sys

import numpy as np

for _p in ("/opt/trn_rl_repo", "/opt/pypackages"):
    if _p not in sys.path:
        sys.path.insert(0, _p)

import ml_dtypes  # noqa: E402

import concourse.bass as bass  # noqa: E402
from concourse import bacc  # noqa: E402
import concourse.mybir as mybir  # noqa: E402
import concourse.tile as tile  # noqa: E402

P = 128
C = 512
HW = 1024
HEADS = 8
DH = 64
HID = 2048
CT = C // P        # 4 channel tiles
HT = HID // P      # 16 hidden tiles
TT = HW // P       # 8 token tiles
NCH = HW // 512    # 2 free-dim chunks of 512
EPS = 1e-6
BF16 = mybir.dt.bfloat16
FP8 = mybir.dt.float8e4
F32 = mybir.dt.float32
AF = mybir.ActivationFunctionType
ALU = mybir.AluOpType

N_CORES = 8
B, H_IMG, W_IMG = 8, 32, 32


# --------------------------------------------------------------------------
# device program
# --------------------------------------------------------------------------

def _emit_proj_one(tc, pools, x_bf, w, out_cf):
    nc = tc.nc
    psum_mm = pools["psum_mm"]
    for ct in range(CT):
        for ch in range(NCH):
            pq = psum_mm.tile([P, 512], F32, tag="mm", name="mm")
            for k in range(CT):
                nc.tensor.matmul(
                    pq,
                    lhsT=w[:, k, ct * P:(ct + 1) * P],
                    rhs=x_bf[:, k, ch * 512:(ch + 1) * 512],
                    start=(k == 0), stop=(k == CT - 1),
                )
            nc.vector.tensor_copy(out=out_cf[:, ct, ch * 512:(ch + 1) * 512], in_=pq)


def _emit_proj_qk(tc, pools, xs_bf, xf_bf, wq, wk, q_cf, k_cf):
    _emit_proj_one(tc, pools, xs_bf, wq, q_cf)
    _emit_proj_one(tc, pools, xf_bf, wk, k_cf)


def _emit_proj_v(tc, pools, xf_bf, wv, v_hf):
    nc = tc.nc
    psum_mm = pools["psum_mm"]
    for tt in range(TT):
        pv = psum_mm.tile([P, 512], F32, tag="mm", name="mm")
        for k in range(CT):
            nc.tensor.matmul(
                pv,
                lhsT=xf_bf[:, k, tt * P:(tt + 1) * P],
                rhs=wv[:, k, :],
                start=(k == 0), stop=(k == CT - 1),
            )
        nc.vector.tensor_copy(
            out=v_hf[:, tt, :, 0:DH],
            in_=pv.rearrange("p (h d) -> p h d", d=DH),
        )
        nc.vector.memset(v_hf[:, tt, :, DH:DH + 1], 1.0)


VW = 72  # V row width: DH + ones col + zero pad (16B-aligned for DoubleRow)


def _emit_st_exp(tc, pools, hp, q_cf, k_cf, filler=None):
    """S^T (row-tiled K=64 pair) -> exp(P^T) in fp8. Returns per-parity PT."""
    nc = tc.nc
    pt = {}
    for par in (0, 1):
        pt[par] = pools["pt"].tile([P, TT, HW], FP8, tag="pt", name="pt", bufs=3)
    ps = {}
    for tt in range(TT):
        if filler is not None:
            filler()
        for par in (0, 1):
            base = par * DH
            p_s = pools["psum_s"].tile([P, HW], F32, tag="s", name="s")
            for ch in range(NCH):
                nc.tensor.matmul(
                    p_s[:, ch * 512:(ch + 1) * 512],
                    lhsT=k_cf[base:base + DH, hp, tt * P:(tt + 1) * P],
                    rhs=q_cf[base:base + DH, hp, ch * 512:(ch + 1) * 512],
                    start=True, stop=True,
                )
            ps[par] = p_s
        for par in (0, 1):
            nc.scalar.activation(out=pt[par][:, tt, :], in_=ps[par],
                                 func=AF.Exp, scale=0.125)
    return pt


def _emit_av(tc, pools, hp, pt, v_hf, o_pair, filler=None):
    """AV+Z (ones column) in fp8 DoubleRow -> normalize into o_pair[:, hp]."""
    nc = tc.nc
    for par in (0, 1):
        h = 2 * hp + par
        for ch in range(NCH):
            if filler is not None:
                filler()
            sl = slice(ch * 512, (ch + 1) * 512)
            pav = pools["psum_av"].tile([VW, 512], F32, tag="av", name="av")
            for tt2 in range(TT // 2):
                nc.tensor.matmul(
                    pav,
                    lhsT=v_hf[:, 2 * tt2:2 * tt2 + 2, h, :],
                    rhs=pt[par][:, 2 * tt2:2 * tt2 + 2, sl],
                    start=(tt2 == 0), stop=(tt2 == TT // 2 - 1),
                    perf_mode=mybir.MatmulPerfMode.DoubleRow,
                )
            rz = pools["rz"].tile([P, 512], F32, tag="rz", name="rz", bufs=2)
            nc.vector.reciprocal(out=rz[DH:DH + 1, :], in_=pav[DH:DH + 1, :])
            nc.sync.dma_start(
                out=rz[0:DH, :],
                in_=rz[DH:DH + 1, None, :].to_broadcast((1, DH, 512)),
            )
            if par == 0:
                nc.vector.tensor_tensor(
                    o_pair[0:DH, hp, sl], pav[0:DH, :], rz[0:DH, :], ALU.mult
                )
            else:
                o_tmp = pools["rz"].tile([DH, 512], FP8, tag="o_tmp",
                                         name="o_tmp", bufs=2)
                nc.vector.tensor_tensor(o_tmp, pav[0:DH, :], rz[0:DH, :], ALU.mult)
                nc.sync.dma_start(out=o_pair[DH:P, hp, sl], in_=o_tmp)


def _emit_layernorm(tc, pools, src_bf, w_ap, b_ap, out_writer, inv512, eps_sb,
                    chunks=tuple(range(NCH)), sub_eng=None):
    """LN over the channel (partition x 4-tile) axis of src_bf [P, CT, HW].

    Pipelined per 512-wide chunk: stats matmuls -> row math -> DMA broadcast
    -> per-ct normalize. out_writer(ct, sl, tile_ap, w, b) consumes each
    normalized [P, 512] piece.
    """
    nc = tc.nc
    psum_mm = pools["psum_mm"]
    for ch in chunks:
        sl = slice(ch * 512, (ch + 1) * 512)
        pmu = psum_mm.tile([1, 512], F32, tag="mm", name="mm")
        for k in range(CT):
            nc.tensor.matmul(
                pmu, lhsT=inv512[:, 0:1], rhs=src_bf[:, k, sl],
                start=(k == 0), stop=(k == CT - 1),
            )
        pms = psum_mm.tile([1, 512], F32, tag="mm", name="mm")
        for k in range(CT):
            r2 = pools["sq"].tile([P, 512], BF16, tag="sq", name="sq")
            nc.gpsimd.tensor_tensor(r2, src_bf[:, k, sl], src_bf[:, k, sl], ALU.mult)
            nc.tensor.matmul(
                pms, lhsT=inv512[:, 0:1], rhs=r2,
                start=(k == 0), stop=(k == CT - 1),
            )
        mu_row = pools["rows"].tile([1, 512], F32, tag="mu_row", name="mu_row", bufs=2)
        rs_row = pools["rows"].tile([1, 512], F32, tag="rs_row", name="rs_row", bufs=2)
        nc.vector.tensor_copy(out=mu_row, in_=pmu)
        musq = pools["rows"].tile([1, 512], F32, tag="musq", name="musq", bufs=1)
        nc.vector.tensor_tensor(musq, mu_row, mu_row, ALU.mult)
        # var = E[x^2] - mu^2 ; rs = 1/sqrt(var + eps)
        nc.vector.tensor_tensor(rs_row, pms, musq, ALU.subtract)
        nc.scalar.activation(rs_row, rs_row, AF.Sqrt, bias=eps_sb[:, 0:1])
        nc.vector.reciprocal(out=rs_row, in_=rs_row)
        mu_b = pools["bcast"].tile([P, 512], F32, tag="mu_b", name="mu_b", bufs=1)
        rs_b = pools["bcast"].tile([P, 512], F32, tag="rs_b", name="rs_b", bufs=1)
        nc.sync.dma_start(out=mu_b, in_=mu_row[0:1, None, :].to_broadcast((1, P, 512)))
        nc.sync.dma_start(out=rs_b, in_=rs_row[0:1, None, :].to_broadcast((1, P, 512)))
        for ct in range(CT):
            tmp = pools["tmp"].tile([P, 512], F32, tag="tmp", name="tmp", bufs=2)
            se = sub_eng if sub_eng is not None else nc.vector
            se.tensor_tensor(tmp, src_bf[:, ct, sl], mu_b, ALU.subtract)
            nc.vector.tensor_tensor(tmp, tmp, rs_b, ALU.mult)
            out_writer(ct, sl, tmp, w_ap(ct), b_ap(ct))


def _emit_wo_residual(tc, pools, pfx, io, cts):
    """Wo projection + bias + residual for the given ct tiles -> r_bf."""
    nc = tc.nc
    o_hf = io["o"]
    x32, wo, params = io["x32"], io["wo"], io["params"]
    psum_mm = pools["psum_mm"]
    if "r" not in io:
        io["r"] = pools["r_pool"].tile([P, CT, HW], BF16, tag=f"r_{pfx}",
                                       name=f"r_{pfx}")
    r_bf = io["r"]
    for ct in cts:
        xr = pools["xr"].tile([P, HW], F32, tag="xr", name="xr")
        nc.sync.dma_start(out=xr, in_=x32[ct * P:(ct + 1) * P, :])
        for ch in range(NCH):
            sl = slice(ch * 512, (ch + 1) * 512)
            pe_ = psum_mm.tile([P, 512], F32, tag="mm", name="mm")
            for i2 in range(HEADS // 4):
                nc.tensor.matmul(
                    pe_,
                    lhsT=wo[:, 2 * i2:2 * i2 + 2, ct * P:(ct + 1) * P],
                    rhs=o_hf[:, 2 * i2:2 * i2 + 2, sl],
                    start=(i2 == 0), stop=(i2 == HEADS // 4 - 1),
                    perf_mode=mybir.MatmulPerfMode.DoubleRow,
                )
            nc.vector.scalar_tensor_tensor(
                out=r_bf[:, ct, sl], in0=pe_, scalar=params["bo"][:, ct:ct + 1],
                in1=xr[:, sl], op0=ALU.add, op1=ALU.add,
            )


def _emit_ln1(tc, pools, pfx, io, chunks=tuple(range(NCH)), sub_eng=None):
    nc = tc.nc
    params = io["params"]
    if "s" not in io:
        io["s"] = pools["s_pool"].tile([P, CT, HW], BF16, tag=f"s_{pfx}",
                                       name=f"s_{pfx}")
    s_bf = io["s"]

    def _ln1_write(ct, sl, tmp, w_scalar, b_scalar):
        nc.vector.tensor_scalar(
            out=s_bf[:, ct, sl], in0=tmp, scalar1=w_scalar, scalar2=b_scalar,
            op0=ALU.mult, op1=ALU.add,
        )

    _emit_layernorm(
        tc, pools, io["r"],
        lambda ct: params["n1w"][:, ct:ct + 1], lambda ct: params["n1b"][:, ct:ct + 1],
        _ln1_write, io["inv512"], io["eps"], chunks, sub_eng=sub_eng,
    )


def _ffn_chunk_pieces(tc, pools, pfx, io, ch):
    """Thunks emitting the FFN chunk piecewise (16 FFN1-ht + 4 FFN2-ct)."""
    nc = tc.nc
    params = io["params"]
    w1, w2 = io["w1"], io["w2"]
    psum_mm = pools["psum_mm"]
    sl = slice(ch * 512, (ch + 1) * 512)
    state = {}

    def ffn1_piece(ht):
        def f():
            if "h" not in state:
                state["h"] = pools["hbuf"].tile([P, HT, 512], BF16, tag="hbuf",
                                                name="hbuf")
            h_ch = state["h"]
            ph = psum_mm.tile([P, 512], F32, tag="mm", name="mm")
            for k in range(CT):
                nc.tensor.matmul(
                    ph,
                    lhsT=w1[:, k, ht * P:(ht + 1) * P],
                    rhs=io["s"][:, k, sl],
                    start=(k == 0), stop=(k == CT - 1),
                )
            nc.scalar.activation(
                out=h_ch[:, ht, :], in_=ph, func=AF.Gelu,
                bias=params["b1"][:, ht:ht + 1],
            )
        return f

    def ffn2_piece(ct):
        def f():
            if "r2" not in io:
                io["r2"] = pools["r_pool"].tile([P, CT, HW], BF16, tag=f"r_{pfx}",
                                                name=f"r2_{pfx}")
            r2_bf = io["r2"]
            h_ch = state["h"]
            pf = psum_mm.tile([P, 512], F32, tag="mm", name="mm")
            for k in range(HT):
                nc.tensor.matmul(
                    pf,
                    lhsT=w2[:, k, ct * P:(ct + 1) * P],
                    rhs=h_ch[:, k, :],
                    start=(k == 0), stop=(k == HT - 1),
                )
            nc.vector.scalar_tensor_tensor(
                out=r2_bf[:, ct, sl], in0=pf, scalar=params["b2"][:, ct:ct + 1],
                in1=io["s"][:, ct, sl], op0=ALU.add, op1=ALU.add,
            )
        return f

    return [ffn1_piece(ht) for ht in range(HT)] + [ffn2_piece(ct) for ct in range(CT)]


def _emit_ffn_chunk(tc, pools, pfx, io, ch):
    """FFN + residual for one 512-wide chunk -> r2_bf."""
    nc = tc.nc
    params = io["params"]
    w1, w2 = io["w1"], io["w2"]
    s_bf = io["s"]
    psum_mm = pools["psum_mm"]
    if "r2" not in io:
        io["r2"] = pools["r_pool"].tile([P, CT, HW], BF16, tag=f"r_{pfx}",
                                        name=f"r2_{pfx}")
    r2_bf = io["r2"]
    sl = slice(ch * 512, (ch + 1) * 512)
    h_ch = pools["hbuf"].tile([P, HT, 512], BF16, tag="hbuf", name="hbuf")
    for ht in range(HT):
        ph = psum_mm.tile([P, 512], F32, tag="mm", name="mm")
        for k in range(CT):
            nc.tensor.matmul(
                ph,
                lhsT=w1[:, k, ht * P:(ht + 1) * P],
                rhs=s_bf[:, k, sl],
                start=(k == 0), stop=(k == CT - 1),
            )
        nc.scalar.activation(
            out=h_ch[:, ht, :], in_=ph, func=AF.Gelu,
            bias=params["b1"][:, ht:ht + 1],
        )
    for ct in range(CT):
        pf = psum_mm.tile([P, 512], F32, tag="mm", name="mm")
        for k in range(HT):
            nc.tensor.matmul(
                pf,
                lhsT=w2[:, k, ct * P:(ct + 1) * P],
                rhs=h_ch[:, k, :],
                start=(k == 0), stop=(k == HT - 1),
            )
        nc.vector.scalar_tensor_tensor(
            out=r2_bf[:, ct, sl], in0=pf, scalar=params["b2"][:, ct:ct + 1],
            in1=s_bf[:, ct, sl], op0=ALU.add, op1=ALU.add,
        )


def _emit_ln2(tc, pools, pfx, io, chunks=tuple(range(NCH)), sub_eng=None):
    nc = tc.nc
    params, out_dram = io["params"], io["out"]

    def _ln2_write(ct, sl, tmp, w_scalar, b_scalar):
        o32 = pools["ostage"].tile([P, 512], F32, tag="ostage", name="ostage", bufs=2)
        nc.vector.tensor_scalar(
            out=o32, in0=tmp, scalar1=w_scalar, scalar2=b_scalar,
            op0=ALU.mult, op1=ALU.add,
        )
        nc.sync.dma_start(out=out_dram[ct * P:(ct + 1) * P, sl], in_=o32)

    _emit_layernorm(
        tc, pools, io["r2"],
        lambda ct: params["n2w"][:, ct:ct + 1], lambda ct: params["n2b"][:, ct:ct + 1],
        _ln2_write, io["inv512"], io["eps"], chunks, sub_eng=sub_eng,
    )


def build_program():
    nc = bacc.Bacc("TRN2", target_bir_lowering=False, debug=False)

    def din(name, shape, dt):
        return nc.dram_tensor(name, list(shape), dt, kind="ExternalInput").ap()

    x32 = {p: din(f"x_{p}32", (C, HW), F32) for p in "sf"}
    xbf = {p: din(f"x_{p}bf", (C, HW), BF16) for p in "sf"}
    wqt = {p: din(f"{p}_wqt", (C, C), BF16) for p in "sf"}
    wkt = {p: din(f"{p}_wkt", (C, C), BF16) for p in "sf"}
    wvt = {p: din(f"{p}_wvt", (C, C), BF16) for p in "sf"}
    wot = {p: din(f"{p}_wot", (C, C), FP8) for p in "sf"}
    w1t = {p: din(f"{p}_w1t", (C, HID), BF16) for p in "sf"}
    w2t = {p: din(f"{p}_w2t", (HID, C), BF16) for p in "sf"}
    pnames = ("bo", "n1w", "n1b", "n2w", "n2b", "b2")
    prm = {
        p: {n: din(f"{p}_{n}", (P, CT), F32) for n in pnames} for p in "sf"
    }
    for p in "sf":
        prm[p]["b1"] = din(f"{p}_b1", (P, HT), F32)
    outs = {
        p: nc.dram_tensor(f"out_{p}", [C, HW], F32, kind="ExternalOutput").ap()
        for p in "sf"
    }

    with tile.TileContext(nc) as tc:
        from contextlib import ExitStack
        with ExitStack() as ctx:
            pools = {}

            def pool(name, bufs, space="SBUF", stack=None):
                pools[name] = (stack or ctx).enter_context(
                    tc.tile_pool(name=name, bufs=bufs, space=space)
                )
                return pools[name]

            # whole-program pools
            pool("psum_mm", 2, space="PSUM")
            pool("psum_s", 2, space="PSUM")
            pool("psum_av", 2, space="PSUM")
            pool("consts", 1)
            pool("params", 1)
            pool("xr", 1)
            pool("rows", 1)
            pool("bcast", 1)
            pool("tmp", 1)
            pool("sq", 2)
            pool("rz", 1)
            pool("pt", 34)
            pool("r_pool", 1)
            pool("s_pool", 1)
            pool("hbuf", 1)
            pool("ostage", 2)
            pool("wffn", 1)

            inv512 = pools["consts"].tile([P, 1], BF16)
            nc.vector.memset(inv512, 1.0 / C)
            eps_sb = pools["consts"].tile([1, 1], F32)
            nc.vector.memset(eps_sb, EPS)

            # ---- load params (small) ----
            params = {}
            for p in "sf":
                params[p] = {}
                for n, ap_ in prm[p].items():
                    t = pools["params"].tile(list(ap_.shape), F32, tag=f"{p}_{n}")
                    nc.sync.dma_start(out=t, in_=ap_)
                    params[p][n] = t

            # ---- pools with manual lifetimes (LIFO discipline) ----
            owo_stack = ctx.enter_context(ExitStack())
            pool("o_pool", 1, stack=owo_stack)
            pool("wo_pool", 1, stack=owo_stack)
            qkv_stack = ctx.enter_context(ExitStack())
            pool("qkv", 1, stack=qkv_stack)
            xw_stack = ctx.enter_context(ExitStack())
            pool("xbf", 1, stack=xw_stack)
            pool("wproj", 1, stack=xw_stack)

            def load_wproj(p, nm, srcw):
                t = pools["wproj"].tile([P, CT, C], BF16, tag=nm, name=f"{nm}_{p}")
                for ct_ in range(CT):
                    eng = (nc.gpsimd, nc.scalar, nc.sync, nc.gpsimd)[ct_ % 4]
                    eng.dma_start(
                        out=t[:, ct_, :], in_=srcw[ct_ * P:(ct_ + 1) * P, :]
                    )
                return t

            def load_xbf(p):
                t = pools["xbf"].tile([P, CT, HW], BF16, tag=f"xbf_{p}",
                                      name=f"xbf_{p}")
                for ct_ in range(CT):
                    eng = (nc.sync, nc.gpsimd, nc.scalar, nc.sync)[ct_ % 4]
                    eng.dma_start(
                        out=t[:, ct_, :], in_=xbf[p][ct_ * P:(ct_ + 1) * P, :]
                    )
                return t

            # Q(s) needs only x_s + wq_s: emit those DMAs first so the first
            # projection matmuls start ~1.3MB into the input stream, not 3.5MB.
            xbf_sb = {"s": load_xbf("s")}
            wq_s = load_wproj("s", "wq", wqt["s"])
            xbf_sb["f"] = load_xbf("f")

            qkv = {}
            for p in "sf":
                qkv[f"q_{p}"] = pools["qkv"].tile(
                    [P, CT, HW], FP8, tag=f"q_{p}", name=f"q_{p}")
                qkv[f"k_{p}"] = pools["qkv"].tile(
                    [P, CT, HW], FP8, tag=f"k_{p}", name=f"k_{p}")
                qkv[f"v_{p}"] = pools["qkv"].tile(
                    [P, TT, HEADS, VW], FP8, tag=f"v_{p}", name=f"v_{p}")
                nc.vector.memset(qkv[f"v_{p}"][:, :, :, DH + 1:], 0.0)

            wo_sb = {}
            o_sb = {}
            for p in "sf":
                wo_sb[p] = pools["wo_pool"].tile([P, CT, C], FP8, tag=f"wo_{p}",
                                                 name=f"wo_{p}")
                o_sb[p] = pools["o_pool"].tile([P, HEADS // 2, HW], FP8,
                                               tag=f"o_{p}", name=f"o_{p}")

            def load_wo(p):
                nc.sync.dma_start(
                    out=wo_sb[p],
                    in_=wot[p].rearrange("(ct p) o -> p ct o", p=P),
                )

            ios = {}
            for p in "sf":
                ios[p] = {
                    "o": o_sb[p], "x32": x32[p], "wo": wo_sb[p],
                    "params": params[p], "out": outs[p],
                    "inv512": inv512, "eps": eps_sb,
                }

            # software-pipelined attention: S^T+exp of pair N overlaps
            # AV of pair N-1 on PE, so PE never waits on the ACT exp chain.
            # stream 's': q from x_s, kv from x_f ; stream 'f': swapped
            seq = [("s", hp) for hp in range(4)] + [("f", hp) for hp in range(4)]
            pts = {}

            def st(i):
                p, hp = seq[i]
                pts[i] = _emit_st_exp(tc, pools, hp, qkv[f"q_{p}"], qkv[f"k_{p}"])

            def av(i):
                p, hp = seq[i]
                _emit_av(tc, pools, hp, pts.pop(i), qkv[f"v_{p}"], o_sb[p])

            # ---- A(s) ----
            _emit_proj_qk(tc, pools, xbf_sb["s"], xbf_sb["f"],
                          wq_s,
                          load_wproj("s", "wk", wkt["s"]),
                          qkv["q_s"], qkv["k_s"])
            _emit_proj_v(tc, pools, xbf_sb["f"], load_wproj("s", "wv", wvt["s"]),
                         qkv["v_s"])

            # ---- B(s) | A(f) ----
            st(0)
            _emit_proj_qk(tc, pools, xbf_sb["f"], xbf_sb["s"],
                          load_wproj("f", "wq", wqt["f"]),
                          load_wproj("f", "wk", wkt["f"]),
                          qkv["q_f"], qkv["k_f"])
            st(1)
            av(0)
            _emit_proj_v(tc, pools, xbf_sb["s"], load_wproj("f", "wv", wvt["f"]),
                         qkv["v_f"])
            load_wo("s")
            st(2)
            av(1)
            load_wo("f")
            st(3)
            av(2)
            xw_stack.close()

            def load_wffn(p):
                t1 = pools["wffn"].tile([P, CT, HID], BF16, tag="w1", name="w1")
                for ct_ in range(CT):
                    eng = (nc.sync, nc.gpsimd, nc.scalar, nc.sync)[ct_ % 4]
                    eng.dma_start(
                        out=t1[:, ct_, :], in_=w1t[p][ct_ * P:(ct_ + 1) * P, :]
                    )
                t2 = pools["wffn"].tile([P, HT, C], BF16, tag="w2", name="w2")
                for g in range(4):
                    eng = (nc.gpsimd, nc.scalar, nc.sync, nc.gpsimd)[g % 4]
                    eng.dma_start(
                        out=t2[:, 4 * g:4 * (g + 1), :],
                        in_=w2t[p][4 * g * P:4 * (g + 1) * P, :].rearrange(
                            "(ht p) o -> p ht o", p=P),
                    )
                return t1, t2

            ios["s"]["w1"], ios["s"]["w2"] = load_wffn("s")

            # ---- B(f) | C(s) | D(s) ----
            st(4)
            av(3)
            _emit_wo_residual(tc, pools, "s", ios["s"], (0, 1))
            st(5)
            av(4)
            _emit_wo_residual(tc, pools, "s", ios["s"], (2, 3))
            st(6)
            av(5)
            _emit_ln1(tc, pools, "s", ios["s"], chunks=(0,))
            st(7)
            av(6)
            _emit_ln1(tc, pools, "s", ios["s"], chunks=(1,))
            _emit_ffn_chunk(tc, pools, "s", ios["s"], 0)
            av(7)
            _emit_ffn_chunk(tc, pools, "s", ios["s"], 1)
            qkv_stack.close()

            # ---- C(f) | LN2(s); then D(f) ----
            _emit_wo_residual(tc, pools, "f", ios["f"], (0, 1))
            _emit_wo_residual(tc, pools, "f", ios["f"], (2, 3))
            _emit_ln1(tc, pools, "f", ios["f"], chunks=(0,))
            _emit_ln2(tc, pools, "s", ios["s"], chunks=(0,), sub_eng=nc.gpsimd)
            _emit_ln1(tc, pools, "f", ios["f"], chunks=(1,))
            ios["f"]["w1"], ios["f"]["w2"] = load_wffn("f")
            _emit_ln2(tc, pools, "s", ios["s"], chunks=(1,), sub_eng=nc.gpsimd)
            _emit_ffn_chunk(tc, pools, "f", ios["f"], 0)
            _emit_ln2(tc, pools, "f", ios["f"], chunks=(0,), sub_eng=nc.gpsimd)
            _emit_ffn_chunk(tc, pools, "f", ios["f"], 1)
            _emit_ln2(tc, pools, "f", ios["f"], chunks=(1,), sub_eng=nc.gpsimd)

    nc.compile()
    return nc


# --------------------------------------------------------------------------
# host side
# --------------------------------------------------------------------------

_BF = ml_dtypes.bfloat16
_F8 = ml_dtypes.float8_e4m3


def _prep_shared_inputs(inputs):
    """Host-side weight prep: transposes, bf16 casts, per-partition layouts."""
    sh = {}
    for p, ap in (("s", "s_"), ("f", "f_")):
        wq, wk, wv, wo = (inputs[ap + n] for n in ("Wq", "Wk", "Wv", "Wo"))
        sh[f"{p}_wqt"] = np.ascontiguousarray(wq.T).astype(_BF)
        sh[f"{p}_wkt"] = np.ascontiguousarray(wk.T).astype(_BF)
        sh[f"{p}_wvt"] = np.ascontiguousarray(wv.T).astype(_BF)
        sh[f"{p}_wot"] = np.ascontiguousarray(wo.T).astype(_F8)
        w1 = inputs[f"{p}ffn_W1"]
        w2 = inputs[f"{p}ffn_W2"]
        sh[f"{p}_w1t"] = np.ascontiguousarray(w1.T).astype(_BF)
        sh[f"{p}_w2t"] = np.ascontiguousarray(w2.T).astype(_BF)
        sh[f"{p}_bo"] = np.ascontiguousarray(
            inputs[ap + "bo"].reshape(CT, P).T
        ).astype(np.float32)
        n1w, n1b = (f"{p}n1_w", f"{p}n1_b")
        n2w, n2b = (f"{p}n2_w", f"{p}n2_b")
        sh[f"{p}_n1w"] = np.ascontiguousarray(inputs[n1w].reshape(CT, P).T).astype(np.float32)
        sh[f"{p}_n1b"] = np.ascontiguousarray(inputs[n1b].reshape(CT, P).T).astype(np.float32)
        sh[f"{p}_n2w"] = np.ascontiguousarray(inputs[n2w].reshape(CT, P).T).astype(np.float32)
        sh[f"{p}_n2b"] = np.ascontiguousarray(inputs[n2b].reshape(CT, P).T).astype(np.float32)
        sh[f"{p}_b1"] = np.ascontiguousarray(
            inputs[f"{p}ffn_b1"].reshape(HT, P).T
        ).astype(np.float32)
        sh[f"{p}_b2"] = np.ascontiguousarray(
            inputs[f"{p}ffn_b2"].reshape(CT, P).T
        ).astype(np.float32)
    return sh


def _rename_ln(inputs):
    """Map reference param names (sn1_w...) onto the scheme used above."""
    out = dict(inputs)
    for p in "sf":
        for i in "12":
            for wb in "wb":
                out[f"{p}n{i}_{wb}"] = inputs[f"{p}n{i}_{wb}"]
    return out


def make_in_maps(inputs):
    inputs = _rename_ln(inputs)
    shared = _prep_shared_inputs(inputs)
    xs = np.ascontiguousarray(inputs["spatial_feat"].reshape(B, C, HW))
    xf = np.ascontiguousarray(inputs["freq_feat"].reshape(B, C, HW))
    in_maps = []
    for b in range(N_CORES):
        m = dict(shared)
        m["x_s32"] = np.ascontiguousarray(xs[b]).astype(np.float32)
        m["x_f32"] = np.ascontiguousarray(xf[b]).astype(np.float32)
        m["x_sbf"] = xs[b].astype(_BF)
        m["x_fbf"] = xf[b].astype(_BF)
        in_maps.append(m)
    return in_maps


_CACHED = {}


def _get_program():
    if "nc" not in _CACHED:
        _CACHED["nc"] = build_program()
    return _CACHED["nc"]


def run_on_hw(inputs, trace=False, trace_kwargs=None):
    from concourse.bass_utils import run_bass_kernel_spmd

    nc = _get_program()
    in_maps = make_in_maps(inputs)
    res = run_bass_kernel_spmd(
        nc, in_maps, list(range(N_CORES)), trace=trace,
        **(dict(trace_kwargs=trace_kwargs) if trace_kwargs else {}),
    )
    s = np.stack([res.results[b]["out_s"] for b in range(B)])
    f = np.stack([res.results[b]["out_f"] for b in range(B)])
    s = s.reshape(B, C, H_IMG, W_IMG).astype(np.float32)
    f = f.reshape(B, C, H_IMG, W_IMG).astype(np.float32)
    return (s, f), res


def kernel(**inputs):
    out, _ = run_on_hw(inputs, trace=False)
    return out



# revision 5
# speedup vs baseline: 1.2542x; 1.2542x over previous
"""CrossFusionBlock Trainium2 kernel.

Dual-stream cross-attention block (B=8, C=512, HW=1024, 8 heads, FFN 2048).
Sharding: data-parallel over batch across 8 NeuronCores (1 batch element per
core), weights replicated. All weight transposes / casts / permutations are
done on the host so the device kernel contains no transposes at all.

Per-core dataflow (channels-first activations):
  Q/K projections (bf16) write head-grouped fp8 tiles q2/k2 [128,(d_hi 2),HW]
    with host-permuted W rows so each head h occupies partitions
    32*(h%4)..32*(h%4)+32 of group g=h//4 with d split (d_lo, d_hi).
  S^T[tk,tq] per head: ONE fp8 DoubleRow matmul (Ki=32 x 2) per 512 chunk.
  P^T = exp(S^T/8) fp8; AV+Z via ones column (fp8 DoubleRow).
  O /= Z via DVE recip + DMA partition-broadcast.
  enh = Wo @ O + bo  (fp8 DoubleRow; Wo host-scaled x16, residual x16 bf16,
    LayerNorm scale-invariance absorbs the 16x).
  LN over channels via PE ones-matmul stats; 1/sqrt(var+eps) computed as
    exp(-0.5*ln(var+eps)) so the softmax Exp table set is reused (Ln and Exp
    share natural_log_exp_and_others; no table reload).
  FFN: fp8 DoubleRow both matmuls, W1/W2 host-scaled x16; gelu descales via
    its scale arg; LN2 absorbs the FFN2-side 16x via s16 = 16*s residual.
"""

import sys

import numpy as np

for _p in ("/opt/trn_rl_repo", "/opt/pypackages"):
    if _p not in sys.path:
        sys.path.insert(0, _p)

import ml_dtypes  # noqa: E402

import concourse.bass as bass  # noqa: E402
from concourse import bacc  # noqa: E402
import concourse.mybir as mybir  # noqa: E402
import concourse.tile as tile  # noqa: E402

P = 128
C = 512
HW = 1024
HEADS = 8
DH = 64
HID = 2048
CT = C // P        # 4 channel tiles
HT = HID // P      # 16 hidden tiles
TT = HW // P       # 8 token tiles
NCH = HW // 512    # 2 free-dim chunks of 512
EPS = 1e-6
BF16 = mybir.dt.bfloat16
FP8 = mybir.dt.float8e4
F32 = mybir.dt.float32
AF = mybir.ActivationFunctionType
ALU = mybir.AluOpType
DR = mybir.MatmulPerfMode.DoubleRow

N_CORES = 8
B, H_IMG, W_IMG = 8, 32, 32

VW = 72  # V row width: DH + ones col + zero pad (16B-aligned for DoubleRow)


# --------------------------------------------------------------------------
# device program
# --------------------------------------------------------------------------

def _emit_proj_qk_one(tc, pools, x_bf, w, out2):
    """Projection with head-grouped permuted W -> out2 = [q2_g0, q2_g1].

    Psum tile t = 2*g + hi holds rows (head 4g+h4, d = 32*hi + lo) at
    partition 32*h4 + lo; evacuated to out2[g][:, hi, :].
    """
    nc = tc.nc
    psum_mm = pools["psum_mm"]
    for t in range(CT):
        g, hi = t // 2, t % 2
        for ch in range(NCH):
            pq = psum_mm.tile([P, 512], F32, tag="mm", name="mm")
            for k in range(CT):
                nc.tensor.matmul(
                    pq,
                    lhsT=w[:, k, t * P:(t + 1) * P],
                    rhs=x_bf[:, k, ch * 512:(ch + 1) * 512],
                    start=(k == 0), stop=(k == CT - 1),
                )
            nc.vector.tensor_copy(
                out=out2[g][:, hi, ch * 512:(ch + 1) * 512], in_=pq
            )


def _emit_proj_v(tc, pools, xf_bf, wv, v_hf):
    nc = tc.nc
    psum_mm = pools["psum_mm"]
    for tt in range(TT):
        pv = psum_mm.tile([P, 512], F32, tag="mm", name="mm")
        for k in range(CT):
            nc.tensor.matmul(
                pv,
                lhsT=xf_bf[:, k, tt * P:(tt + 1) * P],
                rhs=wv[:, k, :],
                start=(k == 0), stop=(k == CT - 1),
            )
        nc.vector.tensor_copy(
            out=v_hf[:, tt, :, 0:DH],
            in_=pv.rearrange("p (h d) -> p h d", d=DH),
        )
        nc.vector.memset(v_hf[:, tt, :, DH:DH + 1], 1.0)


def _emit_st_exp(tc, pools, hp, q2g, k2g, filler=None):
    """S^T per head via fp8 DoubleRow (Ki=32 x2) -> exp(P^T) fp8."""
    nc = tc.nc
    pt = {}
    for par in (0, 1):
        pt[par] = pools["pt"].tile([P, TT, HW], FP8, tag="pt", name="pt", bufs=3)
    ps = {}
    for tt in range(TT):
        if filler is not None:
            filler()
        for par in (0, 1):
            h4 = (2 * hp + par) % 4
            base = 32 * h4
            kw = {"tile_position": (96, 0)} if h4 == 3 else {}
            p_s = pools["psum_s"].tile([P, HW], F32, tag="s", name="s")
            for ch in range(NCH):
                nc.tensor.matmul(
                    p_s[:, ch * 512:(ch + 1) * 512],
                    lhsT=k2g[base:base + 32, :, tt * P:(tt + 1) * P],
                    rhs=q2g[base:base + 32, :, ch * 512:(ch + 1) * 512],
                    start=True, stop=True,
                    perf_mode=DR,
                    **kw,
                )
            ps[par] = p_s
        for par in (0, 1):
            nc.scalar.activation(out=pt[par][:, tt, :], in_=ps[par],
                                 func=AF.Exp, scale=0.125)
    return pt


def _emit_av(tc, pools, hp, pt, v_hf, o_pair, filler=None):
    """AV+Z (ones column) in fp8 DoubleRow -> normalize into o_pair[:, hp]."""
    nc = tc.nc
    for par in (0, 1):
        h = 2 * hp + par
        for ch in range(NCH):
            if filler is not None:
                filler()
            sl = slice(ch * 512, (ch + 1) * 512)
            pav = pools["psum_av"].tile([VW, 512], F32, tag="av", name="av")
            for tt2 in range(TT // 2):
                nc.tensor.matmul(
                    pav,
                    lhsT=v_hf[:, 2 * tt2:2 * tt2 + 2, h, :],
                    rhs=pt[par][:, 2 * tt2:2 * tt2 + 2, sl],
                    start=(tt2 == 0), stop=(tt2 == TT // 2 - 1),
                    perf_mode=DR,
                )
            rz = pools["rz"].tile([P, 512], F32, tag="rz", name="rz", bufs=2)
            nc.vector.reciprocal(out=rz[DH:DH + 1, :], in_=pav[DH:DH + 1, :])
            nc.sync.dma_start(
                out=rz[0:DH, :],
                in_=rz[DH:DH + 1, None, :].to_broadcast((1, DH, 512)),
            )
            if par == 0:
                nc.vector.tensor_tensor(
                    o_pair[0:DH, hp, sl], pav[0:DH, :], rz[0:DH, :], ALU.mult
                )
            else:
                o_tmp = pools["rz"].tile([DH, 512], FP8, tag="o_tmp",
                                         name="o_tmp", bufs=2)
                nc.vector.tensor_tensor(o_tmp, pav[0:DH, :], rz[0:DH, :], ALU.mult)
                nc.sync.dma_start(out=o_pair[DH:P, hp, sl], in_=o_tmp)


def _emit_layernorm(tc, pools, src_bf, out_writer, inv512, eps_sb,
                    chunks=tuple(range(NCH))):
    """LN over the channel (partition x 4-tile) axis of src_bf [P, CT, HW].

    Stats via PE ones-matmul; rs = exp(-0.5*ln(var+eps)) on ACT (stays in
    the Exp table set); mu/rs broadcast to [P,512] bf16 via DMA; normalize
    with bf16 2x DVE tensor ops. out_writer(ct, sl, tmp) consumes each
    normalized (true-scale) [P, 512] bf16 piece.
    """
    nc = tc.nc
    psum_mm = pools["psum_mm"]
    for ch in chunks:
        sl = slice(ch * 512, (ch + 1) * 512)
        pmu = psum_mm.tile([1, 512], F32, tag="mm", name="mm")
        for k in range(CT):
            nc.tensor.matmul(
                pmu, lhsT=inv512[:, 0:1], rhs=src_bf[:, k, sl],
                start=(k == 0), stop=(k == CT - 1),
            )
        pms = psum_mm.tile([1, 512], F32, tag="mm", name="mm")
        for k in range(CT):
            r2 = pools["sq"].tile([P, 512], BF16, tag="sq", name="sq")
            nc.gpsimd.tensor_tensor(r2, src_bf[:, k, sl], src_bf[:, k, sl],
                                    ALU.mult)
            nc.tensor.matmul(
                pms, lhsT=inv512[:, 0:1], rhs=r2,
                start=(k == 0), stop=(k == CT - 1),
            )
        mu_row = pools["rows"].tile([1, 512], BF16, tag="mu_row",
                                    name="mu_row", bufs=2)
        rs_row = pools["rows"].tile([1, 512], BF16, tag="rs_row",
                                    name="rs_row", bufs=2)
        nc.vector.tensor_copy(out=mu_row, in_=pmu)
        musq = pools["rows"].tile([1, 512], F32, tag="musq", name="musq", bufs=1)
        nc.vector.tensor_tensor(musq, mu_row, mu_row, ALU.mult)
        var_row = pools["rows"].tile([1, 512], F32, tag="var_row",
                                     name="var_row", bufs=1)
        nc.vector.tensor_tensor(var_row, pms, musq, ALU.subtract)
        # rs = 1/sqrt(var+eps) = exp(-0.5*ln(var+eps)); Ln+Exp share one
        # activation table set with the softmax Exp -> no table reload.
        lnv = pools["rows"].tile([1, 512], F32, tag="lnv", name="lnv", bufs=1)
        nc.scalar.activation(lnv, var_row, AF.Ln, bias=eps_sb[:, 0:1])
        nc.scalar.activation(rs_row, lnv, AF.Exp, scale=-0.5)
        mu_b = pools["bcast"].tile([P, 512], BF16, tag="mu_b", name="mu_b", bufs=2)
        rs_b = pools["bcast"].tile([P, 512], BF16, tag="rs_b", name="rs_b", bufs=2)
        nc.sync.dma_start(out=mu_b, in_=mu_row[0:1, None, :].to_broadcast((1, P, 512)))
        nc.sync.dma_start(out=rs_b, in_=rs_row[0:1, None, :].to_broadcast((1, P, 512)))
        for ct in range(CT):
            tmp = pools["tmp"].tile([P, 512], BF16, tag="tmp", name="tmp", bufs=2)
            nc.vector.tensor_tensor(tmp, src_bf[:, ct, sl], mu_b, ALU.subtract)
            nc.vector.tensor_tensor(tmp, tmp, rs_b, ALU.mult)
            out_writer(ct, sl, tmp)


def _emit_wo_residual(tc, pools, pfx, io, cts):
    """Wo(16x fp8) projection + 16*bo + 16*x residual -> r_bf (=16*r)."""
    nc = tc.nc
    o_hf = io["o"]
    x16, wo, params = io["x16"], io["wo"], io["params"]
    psum_mm = pools["psum_mm"]
    if "r" not in io:
        io["r"] = pools["r_pool"].tile([P, CT, HW], BF16, tag=f"r_{pfx}",
                                       name=f"r_{pfx}")
    r_bf = io["r"]
    for ct in cts:
        xr = pools["xr"].tile([P, HW], BF16, tag="xr", name="xr", bufs=2)
        nc.sync.dma_start(out=xr, in_=x16[ct * P:(ct + 1) * P, :])
        for ch in range(NCH):
            sl = slice(ch * 512, (ch + 1) * 512)
            pe_ = psum_mm.tile([P, 512], F32, tag="mm", name="mm")
            for i2 in range(HEADS // 4):
                nc.tensor.matmul(
                    pe_,
                    lhsT=wo[:, 2 * i2:2 * i2 + 2, ct * P:(ct + 1) * P],
                    rhs=o_hf[:, 2 * i2:2 * i2 + 2, sl],
                    start=(i2 == 0), stop=(i2 == HEADS // 4 - 1),
                    perf_mode=DR,
                )
            nc.vector.scalar_tensor_tensor(
                out=r_bf[:, ct, sl], in0=pe_, scalar=params["bo16"][:, ct:ct + 1],
                in1=xr[:, sl], op0=ALU.add, op1=ALU.add,
            )


def _emit_ln1(tc, pools, pfx, io, chunks=tuple(range(NCH))):
    nc = tc.nc
    params = io["params"]
    if "s8" not in io:
        io["s8"] = pools["s_pool"].tile([P, CT, HW], FP8, tag=f"s8_{pfx}",
                                        name=f"s8_{pfx}")
        io["s16"] = pools["s_pool"].tile([P, CT, HW], BF16, tag=f"s16_{pfx}",
                                         name=f"s16_{pfx}")
    s8, s16 = io["s8"], io["s16"]

    def _ln1_write(ct, sl, tmp):
        nc.vector.tensor_scalar(
            out=s8[:, ct, sl], in0=tmp,
            scalar1=params["n1w"][:, ct:ct + 1],
            scalar2=params["n1b"][:, ct:ct + 1],
            op0=ALU.mult, op1=ALU.add,
        )
        nc.vector.tensor_scalar(
            out=s16[:, ct, sl], in0=tmp,
            scalar1=params["n1w16"][:, ct:ct + 1],
            scalar2=params["n1b16"][:, ct:ct + 1],
            op0=ALU.mult, op1=ALU.add,
        )

    _emit_layernorm(tc, pools, io["r"], _ln1_write, io["inv512"], io["eps"],
                    chunks)


def _emit_ffn1(tc, pools, pfx, io, hts):
    """FFN1 (fp8 DR, W1 x16) + gelu(scale=1/16) -> h fp8 [P, HT, HW]."""
    nc = tc.nc
    params = io["params"]
    w1 = io["w1"]
    if "h" not in io:
        io["h"] = pools["hbuf"].tile([P, HT, HW], FP8, tag="hbuf", name="hbuf")
    h = io["h"]
    s8 = io["s8"]
    for ht in hts:
        ph = pools["psum_s"].tile([P, HW], F32, tag="s", name="s")
        for ch in range(NCH):
            sl = slice(ch * 512, (ch + 1) * 512)
            for k in range(CT // 2):
                nc.tensor.matmul(
                    ph[:, sl],
                    lhsT=w1[:, 2 * k:2 * k + 2, ht * P:(ht + 1) * P],
                    rhs=s8[:, 2 * k:2 * k + 2, sl],
                    start=(k == 0), stop=(k == CT // 2 - 1),
                    perf_mode=DR,
                )
        nc.scalar.activation(
            out=h[:, ht, :], in_=ph, func=AF.Gelu,
            bias=params["b1"][:, ht:ht + 1], scale=1.0 / 16.0,
        )


def _emit_ffn2(tc, pools, pfx, io, ct_chs):
    """FFN2 (fp8 DR, W2 x16) + 16*b2 + s16 residual -> r2_bf (=16*r2)."""
    nc = tc.nc
    params = io["params"]
    w2 = io["w2"]
    h = io["h"]
    psum_mm = pools["psum_mm"]
    if "r2" not in io:
        io["r2"] = pools["r_pool"].tile([P, CT, HW], BF16, tag=f"r_{pfx}",
                                        name=f"r2_{pfx}")
    r2_bf = io["r2"]
    for ct, ch in ct_chs:
        sl = slice(ch * 512, (ch + 1) * 512)
        pf = psum_mm.tile([P, 512], F32, tag="mm", name="mm")
        for k in range(HT // 2):
            nc.tensor.matmul(
                pf,
                lhsT=w2[:, 2 * k:2 * k + 2, ct * P:(ct + 1) * P],
                rhs=h[:, 2 * k:2 * k + 2, sl],
                start=(k == 0), stop=(k == HT // 2 - 1),
                perf_mode=DR,
            )
        nc.vector.scalar_tensor_tensor(
            out=r2_bf[:, ct, sl], in0=pf, scalar=params["b216"][:, ct:ct + 1],
            in1=io["s16"][:, ct, sl], op0=ALU.add, op1=ALU.add,
        )


def _emit_ln2(tc, pools, pfx, io, chunks=tuple(range(NCH))):
    nc = tc.nc
    params, out_dram = io["params"], io["out"]

    def _ln2_write(ct, sl, tmp):
        o32 = pools["ostage"].tile([P, 512], F32, tag="ostage", name="ostage",
                                   bufs=2)
        nc.vector.tensor_scalar(
            out=o32, in0=tmp,
            scalar1=params["n2w"][:, ct:ct + 1],
            scalar2=params["n2b"][:, ct:ct + 1],
            op0=ALU.mult, op1=ALU.add,
        )
        nc.sync.dma_start(out=out_dram[ct * P:(ct + 1) * P, sl], in_=o32)

    _emit_layernorm(tc, pools, io["r2"], _ln2_write, io["inv512"], io["eps"],
                    chunks)


def build_program():
    nc = bacc.Bacc("TRN2", target_bir_lowering=False, debug=False)

    def din(name, shape, dt):
        return nc.dram_tensor(name, list(shape), dt, kind="ExternalInput").ap()

    x16 = {p: din(f"x_{p}16", (C, HW), BF16) for p in "sf"}
    xbf = {p: din(f"x_{p}bf", (C, HW), BF16) for p in "sf"}
    wqt = {p: din(f"{p}_wqt", (C, C), BF16) for p in "sf"}
    wkt = {p: din(f"{p}_wkt", (C, C), BF16) for p in "sf"}
    wvt = {p: din(f"{p}_wvt", (C, C), BF16) for p in "sf"}
    wot = {p: din(f"{p}_wot", (C, C), FP8) for p in "sf"}
    w1t = {p: din(f"{p}_w1t", (C, HID), FP8) for p in "sf"}
    w2t = {p: din(f"{p}_w2t", (HID, C), FP8) for p in "sf"}
    pnames = ("bo16", "n1w", "n1b", "n1w16", "n1b16", "n2w", "n2b", "b216")
    prm = {
        p: {n: din(f"{p}_{n}", (P, CT), F32) for n in pnames} for p in "sf"
    }
    for p in "sf":
        prm[p]["b1"] = din(f"{p}_b1", (P, HT), F32)
    outs = {
        p: nc.dram_tensor(f"out_{p}", [C, HW], F32, kind="ExternalOutput").ap()
        for p in "sf"
    }

    with tile.TileContext(nc) as tc:
        from contextlib import ExitStack
        with ExitStack() as ctx:
            pools = {}

            def pool(name, bufs, space="SBUF", stack=None):
                pools[name] = (stack or ctx).enter_context(
                    tc.tile_pool(name=name, bufs=bufs, space=space)
                )
                return pools[name]

            # whole-program pools
            pool("psum_mm", 2, space="PSUM")
            pool("psum_s", 2, space="PSUM")
            pool("psum_av", 2, space="PSUM")
            pool("consts", 1)
            pool("params", 1)
            pool("xr", 1)
            pool("rows", 1)
            pool("bcast", 1)
            pool("tmp", 1)
            pool("sq", 2)
            pool("rz", 1)
            pool("pt", 34)
            pool("r_pool", 1)
            pool("s_pool", 1)
            pool("hbuf", 1)
            pool("ostage", 2)
            pool("wffn", 1)

            inv512 = pools["consts"].tile([P, 1], BF16)
            nc.vector.memset(inv512, 1.0 / C)
            eps_sb = pools["consts"].tile([1, 1], F32)
            nc.vector.memset(eps_sb, EPS)
            # Pin the ACT table set to natural_log_exp_and_others (Ln+Exp)
            # before the softmax exps start.
            lnpin = pools["consts"].tile([1, 1], F32)
            nc.vector.memset(lnpin, 1.0)
            nc.scalar.activation(lnpin, lnpin, AF.Ln, bias=eps_sb[:, 0:1])

            # ---- load params (small) ----
            params = {}
            for p in "sf":
                params[p] = {}
                for n, ap_ in prm[p].items():
                    t = pools["params"].tile(list(ap_.shape), F32, tag=f"{p}_{n}")
                    nc.sync.dma_start(out=t, in_=ap_)
                    params[p][n] = t

            # ---- pools with manual lifetimes (LIFO discipline) ----
            owo_stack = ctx.enter_context(ExitStack())
            pool("o_pool", 1, stack=owo_stack)
            pool("wo_pool", 1, stack=owo_stack)
            qkv_stack = ctx.enter_context(ExitStack())
            pool("qkv", 1, stack=qkv_stack)
            xw_stack = ctx.enter_context(ExitStack())
            pool("xbf", 1, stack=xw_stack)
            pool("wproj", 1, stack=xw_stack)

            def load_wproj(p, nm, srcw):
                t = pools["wproj"].tile([P, CT, C], BF16, tag=nm, name=f"{nm}_{p}")
                for ct_ in range(CT):
                    eng = (nc.gpsimd, nc.scalar, nc.sync, nc.gpsimd)[ct_ % 4]
                    eng.dma_start(
                        out=t[:, ct_, :], in_=srcw[ct_ * P:(ct_ + 1) * P, :]
                    )
                return t

            def load_xbf(p):
                t = pools["xbf"].tile([P, CT, HW], BF16, tag=f"xbf_{p}",
                                      name=f"xbf_{p}")
                for ct_ in range(CT):
                    eng = (nc.sync, nc.gpsimd, nc.scalar, nc.sync)[ct_ % 4]
                    eng.dma_start(
                        out=t[:, ct_, :], in_=xbf[p][ct_ * P:(ct_ + 1) * P, :]
                    )
                return t

            # Q(s) needs only x_s + wq_s: emit those DMAs first so the first
            # projection matmuls start early in the input stream.
            xbf_sb = {"s": load_xbf("s")}
            wq_s = load_wproj("s", "wq", wqt["s"])
            xbf_sb["f"] = load_xbf("f")

            qkv = {}
            for p in "sf":
                for g in range(2):
                    qkv[f"q_{p}{g}"] = pools["qkv"].tile(
                        [P, 2, HW], FP8, tag=f"q_{p}{g}", name=f"q_{p}{g}")
                    qkv[f"k_{p}{g}"] = pools["qkv"].tile(
                        [P, 2, HW], FP8, tag=f"k_{p}{g}", name=f"k_{p}{g}")
                qkv[f"v_{p}"] = pools["qkv"].tile(
                    [P, TT, HEADS, VW], FP8, tag=f"v_{p}", name=f"v_{p}")
                nc.vector.memset(qkv[f"v_{p}"][:, :, :, DH + 1:], 0.0)

            wo_sb = {}
            o_sb = {}
            for p in "sf":
                wo_sb[p] = pools["wo_pool"].tile([P, CT, C], FP8, tag=f"wo_{p}",
                                                 name=f"wo_{p}")
                o_sb[p] = pools["o_pool"].tile([P, HEADS // 2, HW], FP8,
                                               tag=f"o_{p}", name=f"o_{p}")

            def load_wo(p):
                nc.sync.dma_start(
                    out=wo_sb[p],
                    in_=wot[p].rearrange("(ct p) o -> p ct o", p=P),
                )

            ios = {}
            for p in "sf":
                ios[p] = {
                    "o": o_sb[p], "x16": x16[p], "wo": wo_sb[p],
                    "params": params[p], "out": outs[p],
                    "inv512": inv512, "eps": eps_sb,
                }

            # software-pipelined attention: S^T+exp of pair N overlaps
            # AV of pair N-1 on PE, so PE never waits on the ACT exp chain.
            # stream 's': q from x_s, kv from x_f ; stream 'f': swapped
            seq = [("s", hp) for hp in range(4)] + [("f", hp) for hp in range(4)]
            pts = {}

            def st(i):
                p, hp = seq[i]
                g = hp // 2
                pts[i] = _emit_st_exp(tc, pools, hp, qkv[f"q_{p}{g}"],
                                      qkv[f"k_{p}{g}"])

            def av(i):
                p, hp = seq[i]
                _emit_av(tc, pools, hp, pts.pop(i), qkv[f"v_{p}"], o_sb[p])

            # ---- A(s) ----
            _emit_proj_qk_one(tc, pools, xbf_sb["s"], wq_s,
                              [qkv["q_s0"], qkv["q_s1"]])
            _emit_proj_qk_one(tc, pools, xbf_sb["f"],
                              load_wproj("s", "wk", wkt["s"]),
                              [qkv["k_s0"], qkv["k_s1"]])
            _emit_proj_v(tc, pools, xbf_sb["f"], load_wproj("s", "wv", wvt["s"]),
                         qkv["v_s"])

            # ---- B(s) | A(f) ----
            st(0)
            _emit_proj_qk_one(tc, pools, xbf_sb["f"],
                              load_wproj("f", "wq", wqt["f"]),
                              [qkv["q_f0"], qkv["q_f1"]])
            st(1)
            av(0)
            _emit_proj_qk_one(tc, pools, xbf_sb["s"],
                              load_wproj("f", "wk", wkt["f"]),
                              [qkv["k_f0"], qkv["k_f1"]])
            st(2)
            av(1)
            _emit_proj_v(tc, pools, xbf_sb["s"], load_wproj("f", "wv", wvt["f"]),
                         qkv["v_f"])
            load_wo("s")
            st(3)
            av(2)
            load_wo("f")
            xw_stack.close()

            def load_wffn(p):
                t1 = pools["wffn"].tile([P, CT, HID], FP8, tag="w1", name="w1")
                for ct_ in range(CT):
                    eng = (nc.sync, nc.gpsimd, nc.scalar, nc.sync)[ct_ % 4]
                    eng.dma_start(
                        out=t1[:, ct_, :], in_=w1t[p][ct_ * P:(ct_ + 1) * P, :]
                    )
                t2 = pools["wffn"].tile([P, HT, C], FP8, tag="w2", name="w2")
                for g in range(4):
                    eng = (nc.gpsimd, nc.scalar, nc.sync, nc.gpsimd)[g % 4]
                    eng.dma_start(
                        out=t2[:, 4 * g:4 * (g + 1), :],
                        in_=w2t[p][4 * g * P:4 * (g + 1) * P, :].rearrange(
                            "(ht p) o -> p ht o", p=P),
                    )
                return t1, t2

            ios["s"]["w1"], ios["s"]["w2"] = load_wffn("s")

            # ---- B(f) | C(s) ----
            st(4)
            av(3)
            _emit_wo_residual(tc, pools, "s", ios["s"], (0, 1))
            st(5)
            av(4)
            _emit_wo_residual(tc, pools, "s", ios["s"], (2, 3))
            st(6)
            av(5)
            _emit_ln1(tc, pools, "s", ios["s"], chunks=(0,))
            st(7)
            av(6)
            _emit_ln1(tc, pools, "s", ios["s"], chunks=(1,))
            av(7)

            # ---- D(s): FFN1(s) gelus queue on ACT after all exps ----
            _emit_ffn1(tc, pools, "s", ios["s"], range(0, 8))
            _emit_wo_residual(tc, pools, "f", ios["f"], (0, 1))
            _emit_ffn1(tc, pools, "s", ios["s"], range(8, 16))
            _emit_wo_residual(tc, pools, "f", ios["f"], (2, 3))
            _emit_ffn2(tc, pools, "s", ios["s"],
                       [(ct, 0) for ct in range(CT)])
            _emit_ln1(tc, pools, "f", ios["f"], chunks=(0,))
            _emit_ffn2(tc, pools, "s", ios["s"],
                       [(ct, 1) for ct in range(CT)])
            _emit_ln1(tc, pools, "f", ios["f"], chunks=(1,))
            qkv_stack.close()

            ios["f"]["w1"], ios["f"]["w2"] = load_wffn("f")

            # ---- LN2(s) | FFN(f); then LN2(f) ----
            _emit_ln2(tc, pools, "s", ios["s"], chunks=(0,))
            _emit_ffn1(tc, pools, "f", ios["f"], range(0, 8))
            _emit_ln2(tc, pools, "s", ios["s"], chunks=(1,))
            _emit_ffn1(tc, pools, "f", ios["f"], range(8, 16))
            _emit_ffn2(tc, pools, "f", ios["f"],
                       [(ct, 0) for ct in range(CT)])
            _emit_ffn2(tc, pools, "f", ios["f"],
                       [(ct, 1) for ct in range(CT)])
            _emit_ln2(tc, pools, "f", ios["f"], chunks=(0,))
            _emit_ln2(tc, pools, "f", ios["f"], chunks=(1,))

    nc.compile()
    return nc


# --------------------------------------------------------------------------
# host side
# --------------------------------------------------------------------------

_BF = ml_dtypes.bfloat16
_F8 = ml_dtypes.float8_e4m3
WS = 16.0  # host weight scale for fp8 matmuls (Wo, W1, W2)


def _head_perm():
    """Permuted output-channel order for Q/K projections.

    Tile t = 2g+hi, partition 32*h4+lo  ->  orig channel (4g+h4)*64+32*hi+lo.
    """
    perm = np.zeros(C, dtype=np.int64)
    i = 0
    for g in range(2):
        for hi in range(2):
            for h4 in range(4):
                for lo in range(32):
                    perm[i] = (4 * g + h4) * 64 + 32 * hi + lo
                    i += 1
    return perm


def _prep_shared_inputs(inputs):
    """Host-side weight prep: transposes, casts, permutations, 16x scales."""
    sh = {}
    perm = _head_perm()
    for p, ap in (("s", "s_"), ("f", "f_")):
        wq, wk, wv, wo = (inputs[ap + n] for n in ("Wq", "Wk", "Wv", "Wo"))
        # wqt columns = permuted W rows (head-grouped d-split layout)
        sh[f"{p}_wqt"] = np.ascontiguousarray(wq.T[:, perm]).astype(_BF)
        sh[f"{p}_wkt"] = np.ascontiguousarray(wk.T[:, perm]).astype(_BF)
        sh[f"{p}_wvt"] = np.ascontiguousarray(wv.T).astype(_BF)
        sh[f"{p}_wot"] = np.ascontiguousarray(wo.T * WS).astype(_F8)
        w1 = inputs[f"{p}ffn_W1"]
        w2 = inputs[f"{p}ffn_W2"]
        sh[f"{p}_w1t"] = np.ascontiguousarray(w1.T * WS).astype(_F8)
        sh[f"{p}_w2t"] = np.ascontiguousarray(w2.T * WS).astype(_F8)
        sh[f"{p}_bo16"] = np.ascontiguousarray(
            (inputs[ap + "bo"] * WS).reshape(CT, P).T
        ).astype(np.float32)
        for nm, key, scale in (
            ("n1w", f"{p}n1_w", 1.0), ("n1b", f"{p}n1_b", 1.0),
            ("n1w16", f"{p}n1_w", WS), ("n1b16", f"{p}n1_b", WS),
            ("n2w", f"{p}n2_w", 1.0), ("n2b", f"{p}n2_b", 1.0),
        ):
            sh[f"{p}_{nm}"] = np.ascontiguousarray(
                (inputs[key] * scale).reshape(CT, P).T
            ).astype(np.float32)
        sh[f"{p}_b1"] = np.ascontiguousarray(
            inputs[f"{p}ffn_b1"].reshape(HT, P).T
        ).astype(np.float32)
        sh[f"{p}_b216"] = np.ascontiguousarray(
            (inputs[f"{p}ffn_b2"] * WS).reshape(CT, P).T
        ).astype(np.float32)
    return sh


def _rename_ln(inputs):
    out = dict(inputs)
    for p in "sf":
        for i in "12":
            for wb in "wb":
                out[f"{p}n{i}_{wb}"] = inputs[f"{p}n{i}_{wb}"]
    return out


def make_in_maps(inputs):
    inputs = _rename_ln(inputs)
    shared = _prep_shared_inputs(inputs)
    xs = np.ascontiguousarray(inputs["spatial_feat"].reshape(B, C, HW))
    xf = np.ascontiguousarray(inputs["freq_feat"].reshape(B, C, HW))
    in_maps = []
    for b in range(N_CORES):
        m = dict(shared)
        m["x_s16"] = (xs[b] * WS).astype(_BF)
        m["x_f16"] = (xf[b] * WS).astype(_BF)
        m["x_sbf"] = xs[b].astype(_BF)
        m["x_fbf"] = xf[b].astype(_BF)
        in_maps.append(m)
    return in_maps


_CACHED = {}


def _get_program():
    if "nc" not in _CACHED:
        _CACHED["nc"] = build_program()
    return _CACHED["nc"]


def run_on_hw(inputs, trace=False, trace_kwargs=None):
    from concourse.bass_utils import run_bass_kernel_spmd

    nc = _get_program()
    in_maps = make_in_maps(inputs)
    res = run_bass_kernel_spmd(
        nc, in_maps, list(range(N_CORES)), trace=trace,
        **(dict(trace_kwargs=trace_kwargs) if trace_kwargs else {}),
    )
    s = np.stack([res.results[b]["out_s"] for b in range(B)])
    f = np.stack([res.results[b]["out_f"] for b in range(B)])
    s = s.reshape(B, C, H_IMG, W_IMG).astype(np.float32)
    f = f.reshape(B, C, H_IMG, W_IMG).astype(np.float32)
    return (s, f), res


def kernel(**inputs):
    out, _ = run_on_hw(inputs, trace=False)
    return out


# revision 15
# speedup vs baseline: 1.2560x; 1.0015x over previous
"""CrossFusionBlock Trainium2 kernel.

Dual-stream cross-attention block (B=8, C=512, HW=1024, 8 heads, FFN 2048).
Sharding: data-parallel over batch across 8 NeuronCores (1 batch element per
core), weights replicated. All weight transposes / casts / permutations are
done on the host so the device kernel contains no transposes at all.

Per-core dataflow (channels-first activations):
  Q/K projections (bf16) write head-grouped fp8 tiles q2/k2 [128,(d_hi 2),HW]
    with host-permuted W rows so each head h occupies partitions
    32*(h%4)..32*(h%4)+32 of group g=h//4 with d split (d_lo, d_hi).
  S^T[tk,tq] per head: ONE fp8 DoubleRow matmul (Ki=32 x 2) per 512 chunk.
  P^T = exp(S^T/8) fp8; AV+Z via ones column (fp8 DoubleRow).
  O /= Z via DVE recip + DMA partition-broadcast.
  enh = Wo @ O + bo  (fp8 DoubleRow; Wo host-scaled x16, residual x16 bf16,
    LayerNorm scale-invariance absorbs the 16x).
  LN over channels via PE ones-matmul stats; 1/sqrt(var+eps) computed as
    exp(-0.5*ln(var+eps)) so the softmax Exp table set is reused (Ln and Exp
    share natural_log_exp_and_others; no table reload).
  FFN: fp8 DoubleRow both matmuls, W1/W2 host-scaled x16; gelu descales via
    its scale arg; LN2 absorbs the FFN2-side 16x via s16 = 16*s residual.
"""

import sys

import numpy as np

for _p in ("/opt/trn_rl_repo", "/opt/pypackages"):
    if _p not in sys.path:
        sys.path.insert(0, _p)

import ml_dtypes  # noqa: E402

import concourse.bass as bass  # noqa: E402
from concourse import bacc  # noqa: E402
import concourse.mybir as mybir  # noqa: E402
import concourse.tile as tile  # noqa: E402


def _patch_act_tables():
    """Make natural_log_exp_and_others the only set offering Exp/Ln.

    The table-load pass greedily picks the first set containing each
    activation function, which ping-pongs between the ln-only and exp-only
    sets (2 table loads per LayerNorm rsqrt). Hiding Exp/Ln from the other
    sets forces the combined set; set ids keep their true act_info indices
    so the emitted LoadActFuncSet ids stay valid for walrus.
    """
    import concourse.hw_specs as hw_specs

    if getattr(hw_specs, "_act_tables_patched", False):
        return
    orig = hw_specs.get_activation_tables

    def patched(arch):
        tabs = dict(orig(arch))
        exp = mybir.ActivationFunctionType.Exp
        ln = mybir.ActivationFunctionType.Ln
        out = {}
        for name, funcs in tabs.items():
            if name != "natural_log_exp_and_others":
                funcs = funcs - {exp, ln}
            out[name] = funcs
        return out

    hw_specs._act_tables_patched = True
    hw_specs.get_activation_tables = patched
    bacc.get_activation_tables = patched


_patch_act_tables()

P = 128
C = 512
HW = 1024
HEADS = 8
DH = 64
HID = 2048
CT = C // P        # 4 channel tiles
HT = HID // P      # 16 hidden tiles
TT = HW // P       # 8 token tiles
NCH = HW // 512    # 2 free-dim chunks of 512
EPS = 1e-6
BF16 = mybir.dt.bfloat16
FP8 = mybir.dt.float8e4
F32 = mybir.dt.float32
AF = mybir.ActivationFunctionType
ALU = mybir.AluOpType
DR = mybir.MatmulPerfMode.DoubleRow

N_CORES = 8
B, H_IMG, W_IMG = 8, 32, 32

VW = 72  # V row width: DH + ones col + zero pad (16B-aligned for DoubleRow)


# --------------------------------------------------------------------------
# device program
# --------------------------------------------------------------------------

def _emit_proj_qk_one(tc, pools, x_bf, w, out2):
    """Projection with head-grouped permuted W -> out2 = [q2_g0, q2_g1].

    Psum tile t = 2*g + hi holds rows (head 4g+h4, d = 32*hi + lo) at
    partition 32*h4 + lo; evacuated to out2[g][:, hi, :].
    """
    nc = tc.nc
    psum_mm = pools["psum_mm"]
    for t in range(CT):
        g, hi = t // 2, t % 2
        for ch in range(NCH):
            pq = psum_mm.tile([P, 512], F32, tag="mm", name="mm")
            for k in range(CT):
                nc.tensor.matmul(
                    pq,
                    lhsT=w[:, k, t * P:(t + 1) * P],
                    rhs=x_bf[:, k, ch * 512:(ch + 1) * 512],
                    start=(k == 0), stop=(k == CT - 1),
                )
            nc.vector.tensor_copy(
                out=out2[g][:, hi, ch * 512:(ch + 1) * 512], in_=pq
            )


def _emit_proj_v(tc, pools, xf_bf, wv, v_hf):
    nc = tc.nc
    psum_mm = pools["psum_mm"]
    for tt in range(TT):
        pv = psum_mm.tile([P, 512], F32, tag="mm", name="mm")
        for k in range(CT):
            nc.tensor.matmul(
                pv,
                lhsT=xf_bf[:, k, tt * P:(tt + 1) * P],
                rhs=wv[:, k, :],
                start=(k == 0), stop=(k == CT - 1),
            )
        nc.vector.tensor_copy(
            out=v_hf[:, tt, :, 0:DH],
            in_=pv.rearrange("p (h d) -> p h d", d=DH),
        )
        nc.vector.memset(v_hf[:, tt, :, DH:DH + 1], 1.0)


def _emit_st_exp(tc, pools, hp, q2g, k2g, filler=None):
    """S^T per head via fp8 DoubleRow (Ki=32 x2) -> exp(P^T) fp8."""
    nc = tc.nc
    pt = {}
    for par in (0, 1):
        pt[par] = pools["pt"].tile([P, TT, HW], FP8, tag="pt", name="pt", bufs=2)
    ps = {}
    for tt in range(TT):
        if filler is not None:
            filler()
        for par in (0, 1):
            h4 = (2 * hp + par) % 4
            base = 32 * h4
            kw = {"tile_position": (96, 0)} if h4 == 3 else {}
            p_s = pools["psum_s"].tile([P, HW], F32, tag="s", name="s")
            for ch in range(NCH):
                nc.tensor.matmul(
                    p_s[:, ch * 512:(ch + 1) * 512],
                    lhsT=k2g[base:base + 32, :, tt * P:(tt + 1) * P],
                    rhs=q2g[base:base + 32, :, ch * 512:(ch + 1) * 512],
                    start=True, stop=True,
                    perf_mode=DR,
                    **kw,
                )
            ps[par] = p_s
        for par in (0, 1):
            nc.scalar.activation(out=pt[par][:, tt, :], in_=ps[par],
                                 func=AF.Exp, scale=0.125)
    return pt


def _emit_av(tc, pools, hp, pt, v_hf, o_pair, filler=None):
    """AV+Z (ones column) in fp8 DoubleRow -> normalize into o_pair[:, hp]."""
    nc = tc.nc
    for par in (0, 1):
        h = 2 * hp + par
        for ch in range(NCH):
            if filler is not None:
                filler()
            sl = slice(ch * 512, (ch + 1) * 512)
            pav = pools["psum_av"].tile([VW, 512], F32, tag="av", name="av")
            for tt2 in range(TT // 2):
                nc.tensor.matmul(
                    pav,
                    lhsT=v_hf[:, 2 * tt2:2 * tt2 + 2, h, :],
                    rhs=pt[par][:, 2 * tt2:2 * tt2 + 2, sl],
                    start=(tt2 == 0), stop=(tt2 == TT // 2 - 1),
                    perf_mode=DR,
                )
            rz = pools["rz"].tile([P, 512], F32, tag="rz", name="rz", bufs=2)
            nc.vector.reciprocal(out=rz[DH:DH + 1, :], in_=pav[DH:DH + 1, :])
            nc.sync.dma_start(
                out=rz[0:DH, :],
                in_=rz[DH:DH + 1, None, :].to_broadcast((1, DH, 512)),
            )
            if par == 0:
                nc.vector.tensor_tensor(
                    o_pair[0:DH, hp, sl], pav[0:DH, :], rz[0:DH, :], ALU.mult
                )
            else:
                o_tmp = pools["rz"].tile([DH, 512], FP8, tag="o_tmp",
                                         name="o_tmp", bufs=2)
                nc.vector.tensor_tensor(o_tmp, pav[0:DH, :], rz[0:DH, :], ALU.mult)
                nc.sync.dma_start(out=o_pair[DH:P, hp, sl], in_=o_tmp)


def _emit_layernorm(tc, pools, src_bf, out_writer, inv512, eps_sb,
                    chunks=tuple(range(NCH))):
    """LN over the channel (partition x 4-tile) axis of src_bf [P, CT, HW].

    Stats via PE ones-matmul; rs = exp(-0.5*ln(var+eps)) on ACT (stays in
    the Exp table set); mu/rs broadcast to [P,512] bf16 via DMA; normalize
    with bf16 2x DVE tensor ops. out_writer(ct, sl, tmp) consumes each
    normalized (true-scale) [P, 512] bf16 piece.
    """
    nc = tc.nc
    psum_mm = pools["psum_mm"]
    mu2 = pools["rows"].tile([1, NCH, 512], BF16, tag="mu2", name="mu2", bufs=2)
    var2 = pools["rows"].tile([1, NCH, 512], F32, tag="var2", name="var2", bufs=2)
    rs2 = pools["rows"].tile([1, NCH, 512], BF16, tag="rs2", name="rs2", bufs=2)
    for ch in chunks:
        sl = slice(ch * 512, (ch + 1) * 512)
        pmu = psum_mm.tile([1, 512], F32, tag="mm", name="mm")
        for k in range(CT):
            nc.tensor.matmul(
                pmu, lhsT=inv512[:, 0:1], rhs=src_bf[:, k, sl],
                start=(k == 0), stop=(k == CT - 1),
            )
        pms = psum_mm.tile([1, 512], F32, tag="mm", name="mm")
        for k in range(CT):
            r2 = pools["sq"].tile([P, 512], BF16, tag="sq", name="sq")
            nc.gpsimd.tensor_tensor(r2, src_bf[:, k, sl], src_bf[:, k, sl],
                                    ALU.mult)
            nc.tensor.matmul(
                pms, lhsT=inv512[:, 0:1], rhs=r2,
                start=(k == 0), stop=(k == CT - 1),
            )
        nc.vector.tensor_copy(out=mu2[0:1, ch, :], in_=pmu)
        musq = pools["rows"].tile([1, 512], F32, tag="musq", name="musq", bufs=2)
        nc.vector.tensor_tensor(musq, mu2[0:1, ch, :], mu2[0:1, ch, :],
                                ALU.mult)
        nc.vector.tensor_tensor(var2[0:1, ch, :], pms, musq, ALU.subtract)
    # rs = 1/sqrt(var+eps) = exp(-0.5*ln(var+eps)); Ln+Exp share one
    # activation table set with the softmax Exp -> no table reload. Both
    # chunks' rows are batched into single [2,512] ACT ops.
    lnv = pools["rows"].tile([1, NCH, 512], F32, tag="lnv", name="lnv", bufs=2)
    nc.scalar.activation(lnv, var2, AF.Ln, bias=eps_sb[:, 0:1])
    nc.scalar.activation(rs2, lnv, AF.Exp, scale=-0.5)
    for ch in chunks:
        sl = slice(ch * 512, (ch + 1) * 512)
        mu_b = pools["bcast"].tile([P, 512], BF16, tag="mu_b", name="mu_b", bufs=2)
        rs_b = pools["bcast"].tile([P, 512], BF16, tag="rs_b", name="rs_b", bufs=2)
        nc.sync.dma_start(out=mu_b, in_=mu2[0:1, ch, None, :].to_broadcast((1, P, 512)))
        nc.sync.dma_start(out=rs_b, in_=rs2[0:1, ch, None, :].to_broadcast((1, P, 512)))
        for ct in range(CT):
            tmp = pools["tmp"].tile([P, 512], BF16, tag="tmp", name="tmp", bufs=2)
            nc.vector.tensor_tensor(tmp, src_bf[:, ct, sl], mu_b, ALU.subtract)
            nc.vector.tensor_tensor(tmp, tmp, rs_b, ALU.mult)
            out_writer(ct, sl, tmp)


def _emit_wo_residual(tc, pools, pfx, io, cts):
    """Wo(16x fp8) projection + 16*bo + 16*x residual -> r_bf (=16*r)."""
    nc = tc.nc
    o_hf = io["o"]
    x16, wo, params = io["x16"], io["wo"], io["params"]
    psum_mm = pools["psum_mm"]
    if "r" not in io:
        io["r"] = pools["r_pool"].tile([P, CT, HW], BF16, tag=f"r_{pfx}",
                                       name=f"r_{pfx}")
    r_bf = io["r"]
    for ct in cts:
        xr = pools["xr"].tile([P, HW], BF16, tag="xr", name="xr", bufs=2)
        nc.sync.dma_start(out=xr, in_=x16[ct * P:(ct + 1) * P, :])
        for ch in range(NCH):
            sl = slice(ch * 512, (ch + 1) * 512)
            pe_ = psum_mm.tile([P, 512], F32, tag="mm", name="mm")
            for i2 in range(HEADS // 4):
                nc.tensor.matmul(
                    pe_,
                    lhsT=wo[:, 2 * i2:2 * i2 + 2, ct * P:(ct + 1) * P],
                    rhs=o_hf[:, 2 * i2:2 * i2 + 2, sl],
                    start=(i2 == 0), stop=(i2 == HEADS // 4 - 1),
                    perf_mode=DR,
                )
            nc.vector.scalar_tensor_tensor(
                out=r_bf[:, ct, sl], in0=pe_, scalar=params["bo16"][:, ct:ct + 1],
                in1=xr[:, sl], op0=ALU.add, op1=ALU.add,
            )


def _emit_ln1(tc, pools, pfx, io, chunks=tuple(range(NCH))):
    nc = tc.nc
    params = io["params"]
    if "s8" not in io:
        io["s8"] = pools["s_pool"].tile([P, CT, HW], FP8, tag=f"s8_{pfx}",
                                        name=f"s8_{pfx}")
        io["s16"] = pools["s_pool"].tile([P, CT, HW], BF16, tag=f"s16_{pfx}",
                                         name=f"s16_{pfx}")
    s8, s16 = io["s8"], io["s16"]

    def _ln1_write(ct, sl, tmp):
        nc.vector.tensor_scalar(
            out=s8[:, ct, sl], in0=tmp,
            scalar1=params["n1w"][:, ct:ct + 1],
            scalar2=params["n1b"][:, ct:ct + 1],
            op0=ALU.mult, op1=ALU.add,
        )
        nc.vector.tensor_scalar(
            out=s16[:, ct, sl], in0=tmp,
            scalar1=params["n1w16"][:, ct:ct + 1],
            scalar2=params["n1b16"][:, ct:ct + 1],
            op0=ALU.mult, op1=ALU.add,
        )

    _emit_layernorm(tc, pools, io["r"], _ln1_write, io["inv512"], io["eps"],
                    chunks)


def _emit_ffn1(tc, pools, pfx, io, hts):
    """FFN1 (fp8 DR, W1 x16) + gelu(scale=1/16) -> h fp8 [P, HT, HW]."""
    nc = tc.nc
    params = io["params"]
    w1 = io["w1"]
    if "h" not in io:
        io["h"] = pools["hbuf"].tile([P, HT, HW], FP8, tag="hbuf",
                                     name="hbuf", bufs=2)
    h = io["h"]
    s8 = io["s8"]
    for ht in hts:
        ph = pools["psum_s"].tile([P, HW], F32, tag="s", name="s")
        for ch in range(NCH):
            sl = slice(ch * 512, (ch + 1) * 512)
            for k in range(CT // 2):
                nc.tensor.matmul(
                    ph[:, sl],
                    lhsT=w1[:, 2 * k:2 * k + 2, ht * P:(ht + 1) * P],
                    rhs=s8[:, 2 * k:2 * k + 2, sl],
                    start=(k == 0), stop=(k == CT // 2 - 1),
                    perf_mode=DR,
                )
        nc.scalar.activation(
            out=h[:, ht, :], in_=ph, func=AF.Gelu,
            bias=params["b1"][:, ht:ht + 1], scale=1.0 / 16.0,
        )


def _emit_ffn2(tc, pools, pfx, io, ct_chs):
    """FFN2 (fp8 DR, W2 x16) + 16*b2 + s16 residual -> r2_bf (=16*r2)."""
    nc = tc.nc
    params = io["params"]
    w2 = io["w2"]
    h = io["h"]
    psum_mm = pools["psum_mm"]
    if "r2" not in io:
        io["r2"] = pools["r_pool"].tile([P, CT, HW], BF16, tag=f"r_{pfx}",
                                        name=f"r2_{pfx}")
    r2_bf = io["r2"]
    for ct, ch in ct_chs:
        sl = slice(ch * 512, (ch + 1) * 512)
        pf = psum_mm.tile([P, 512], F32, tag="mm", name="mm")
        for k in range(HT // 2):
            nc.tensor.matmul(
                pf,
                lhsT=w2[:, 2 * k:2 * k + 2, ct * P:(ct + 1) * P],
                rhs=h[:, 2 * k:2 * k + 2, sl],
                start=(k == 0), stop=(k == HT // 2 - 1),
                perf_mode=DR,
            )
        nc.vector.scalar_tensor_tensor(
            out=r2_bf[:, ct, sl], in0=pf, scalar=params["b216"][:, ct:ct + 1],
            in1=io["s16"][:, ct, sl], op0=ALU.add, op1=ALU.add,
        )


def _emit_ln2(tc, pools, pfx, io, chunks=tuple(range(NCH))):
    nc = tc.nc
    params, out_dram = io["params"], io["out"]

    def _ln2_write(ct, sl, tmp):
        o32 = pools["ostage"].tile([P, 512], F32, tag="ostage", name="ostage",
                                   bufs=2)
        nc.vector.tensor_scalar(
            out=o32, in0=tmp,
            scalar1=params["n2w"][:, ct:ct + 1],
            scalar2=params["n2b"][:, ct:ct + 1],
            op0=ALU.mult, op1=ALU.add,
        )
        nc.sync.dma_start(out=out_dram[ct * P:(ct + 1) * P, sl], in_=o32)

    _emit_layernorm(tc, pools, io["r2"], _ln2_write, io["inv512"], io["eps"],
                    chunks)


def build_program():
    nc = bacc.Bacc("TRN2", target_bir_lowering=False, debug=False)

    def din(name, shape, dt):
        return nc.dram_tensor(name, list(shape), dt, kind="ExternalInput").ap()

    x16 = {p: din(f"x_{p}16", (C, HW), BF16) for p in "sf"}
    xbf = {p: din(f"x_{p}bf", (C, HW), BF16) for p in "sf"}
    wqt = {p: din(f"{p}_wqt", (C, C), BF16) for p in "sf"}
    wkt = {p: din(f"{p}_wkt", (C, C), BF16) for p in "sf"}
    wvt = {p: din(f"{p}_wvt", (C, C), BF16) for p in "sf"}
    wot = {p: din(f"{p}_wot", (C, C), FP8) for p in "sf"}
    w1t = {p: din(f"{p}_w1t", (C, HID), FP8) for p in "sf"}
    w2t = {p: din(f"{p}_w2t", (HID, C), FP8) for p in "sf"}
    pnames = ("bo16", "n1w", "n1b", "n1w16", "n1b16", "n2w", "n2b", "b216")
    prm = {
        p: {n: din(f"{p}_{n}", (P, CT), F32) for n in pnames} for p in "sf"
    }
    for p in "sf":
        prm[p]["b1"] = din(f"{p}_b1", (P, HT), F32)
    outs = {
        p: nc.dram_tensor(f"out_{p}", [C, HW], F32, kind="ExternalOutput").ap()
        for p in "sf"
    }

    with tile.TileContext(nc) as tc:
        from contextlib import ExitStack
        with ExitStack() as ctx:
            pools = {}

            def pool(name, bufs, space="SBUF", stack=None):
                pools[name] = (stack or ctx).enter_context(
                    tc.tile_pool(name=name, bufs=bufs, space=space)
                )
                return pools[name]

            # whole-program pools
            pool("psum_mm", 2, space="PSUM")
            pool("psum_s", 2, space="PSUM")
            pool("psum_av", 2, space="PSUM")
            pool("consts", 1)
            pool("params", 1)
            pool("xr", 1)
            pool("rows", 1)
            pool("bcast", 1)
            pool("tmp", 1)
            pool("sq", 2)
            pool("rz", 1)
            pool("pt", 34)
            pool("r_pool", 1)
            pool("s_pool", 1)
            pool("ostage", 2)

            inv512 = pools["consts"].tile([P, 1], BF16)
            nc.vector.memset(inv512, 1.0 / C)
            eps_sb = pools["consts"].tile([1, 1], F32)
            nc.vector.memset(eps_sb, EPS)
            # Pin the ACT table set to natural_log_exp_and_others (Ln+Exp)
            # before the softmax exps start.
            lnpin = pools["consts"].tile([1, 1], F32)
            nc.vector.memset(lnpin, 1.0)
            nc.scalar.activation(lnpin, lnpin, AF.Ln, bias=eps_sb[:, 0:1])

            # ---- load params (small) ----
            params = {}
            for p in "sf":
                params[p] = {}
                for n, ap_ in prm[p].items():
                    t = pools["params"].tile(list(ap_.shape), F32, tag=f"{p}_{n}")
                    nc.sync.dma_start(out=t, in_=ap_)
                    params[p][n] = t

            # ---- pools with manual lifetimes (LIFO discipline) ----
            owo_stack = ctx.enter_context(ExitStack())
            pool("o_pool", 1, stack=owo_stack)
            pool("wo_pool", 1, stack=owo_stack)
            qkv_stack = ctx.enter_context(ExitStack())
            pool("qkv", 1, stack=qkv_stack)
            xw_stack = ctx.enter_context(ExitStack())
            pool("xbf", 1, stack=xw_stack)
            pool("wproj", 1, stack=xw_stack)

            def load_wproj(p, nm, srcw):
                t = pools["wproj"].tile([P, CT, C], BF16, tag=nm, name=f"{nm}_{p}")
                for ct_ in range(CT):
                    eng = (nc.gpsimd, nc.scalar, nc.sync, nc.gpsimd)[ct_ % 4]
                    eng.dma_start(
                        out=t[:, ct_, :], in_=srcw[ct_ * P:(ct_ + 1) * P, :]
                    )
                return t

            def load_xbf(p):
                t = pools["xbf"].tile([P, CT, HW], BF16, tag=f"xbf_{p}",
                                      name=f"xbf_{p}")
                for ct_ in range(CT):
                    eng = (nc.sync, nc.gpsimd, nc.scalar, nc.sync)[ct_ % 4]
                    eng.dma_start(
                        out=t[:, ct_, :], in_=xbf[p][ct_ * P:(ct_ + 1) * P, :]
                    )
                return t

            # Q(s) needs only x_s + wq_s: emit those DMAs first so the first
            # projection matmuls start early in the input stream.
            xbf_sb = {"s": load_xbf("s")}
            wq_s = load_wproj("s", "wq", wqt["s"])
            xbf_sb["f"] = load_xbf("f")

            qkv = {}
            for p in "sf":
                for g in range(2):
                    qkv[f"q_{p}{g}"] = pools["qkv"].tile(
                        [P, 2, HW], FP8, tag=f"q_{p}{g}", name=f"q_{p}{g}")
                    qkv[f"k_{p}{g}"] = pools["qkv"].tile(
                        [P, 2, HW], FP8, tag=f"k_{p}{g}", name=f"k_{p}{g}")
                qkv[f"v_{p}"] = pools["qkv"].tile(
                    [P, TT, HEADS, VW], FP8, tag=f"v_{p}", name=f"v_{p}")
                nc.vector.memset(qkv[f"v_{p}"][:, :, :, DH + 1:], 0.0)

            wo_sb = {}
            o_sb = {}
            for p in "sf":
                wo_sb[p] = pools["wo_pool"].tile([P, CT, C], FP8, tag=f"wo_{p}",
                                                 name=f"wo_{p}")
                o_sb[p] = pools["o_pool"].tile([P, HEADS // 2, HW], FP8,
                                               tag=f"o_{p}", name=f"o_{p}")

            def load_wo(p):
                nc.sync.dma_start(
                    out=wo_sb[p],
                    in_=wot[p].rearrange("(ct p) o -> p ct o", p=P),
                )

            ios = {}
            for p in "sf":
                ios[p] = {
                    "o": o_sb[p], "x16": x16[p], "wo": wo_sb[p],
                    "params": params[p], "out": outs[p],
                    "inv512": inv512, "eps": eps_sb,
                }

            # software-pipelined attention: S^T+exp of pair N overlaps
            # AV of pair N-1 on PE, so PE never waits on the ACT exp chain.
            # stream 's': q from x_s, kv from x_f ; stream 'f': swapped
            seq = [("s", hp) for hp in range(4)] + [("f", hp) for hp in range(4)]
            pts = {}

            def st(i):
                p, hp = seq[i]
                g = hp // 2
                pts[i] = _emit_st_exp(tc, pools, hp, qkv[f"q_{p}{g}"],
                                      qkv[f"k_{p}{g}"])

            def av(i):
                p, hp = seq[i]
                _emit_av(tc, pools, hp, pts.pop(i), qkv[f"v_{p}"], o_sb[p])

            # ---- A(s) ----
            _emit_proj_qk_one(tc, pools, xbf_sb["s"], wq_s,
                              [qkv["q_s0"], qkv["q_s1"]])
            _emit_proj_qk_one(tc, pools, xbf_sb["f"],
                              load_wproj("s", "wk", wkt["s"]),
                              [qkv["k_s0"], qkv["k_s1"]])
            _emit_proj_v(tc, pools, xbf_sb["f"], load_wproj("s", "wv", wvt["s"]),
                         qkv["v_s"])

            # ---- B(s) | A(f) ----
            st(0)
            _emit_proj_qk_one(tc, pools, xbf_sb["f"],
                              load_wproj("f", "wq", wqt["f"]),
                              [qkv["q_f0"], qkv["q_f1"]])
            st(1)
            av(0)
            _emit_proj_qk_one(tc, pools, xbf_sb["s"],
                              load_wproj("f", "wk", wkt["f"]),
                              [qkv["k_f0"], qkv["k_f1"]])
            st(2)
            av(1)
            _emit_proj_v(tc, pools, xbf_sb["s"], load_wproj("f", "wv", wvt["f"]),
                         qkv["v_f"])
            load_wo("s")
            st(3)
            av(2)
            load_wo("f")
            xw_stack.close()
            # wffn/hbuf reuse the SBUF space freed by xbf/wproj above.
            pool("wffn", 1)
            pool("hbuf", 1)

            def load_wffn(p):
                t1 = pools["wffn"].tile([P, CT, HID], FP8, tag="w1", name="w1")
                for ct_ in range(CT):
                    eng = (nc.sync, nc.gpsimd, nc.scalar, nc.sync)[ct_ % 4]
                    eng.dma_start(
                        out=t1[:, ct_, :], in_=w1t[p][ct_ * P:(ct_ + 1) * P, :]
                    )
                t2 = pools["wffn"].tile([P, HT, C], FP8, tag="w2", name="w2")
                for g in range(4):
                    eng = (nc.gpsimd, nc.scalar, nc.sync, nc.gpsimd)[g % 4]
                    eng.dma_start(
                        out=t2[:, 4 * g:4 * (g + 1), :],
                        in_=w2t[p][4 * g * P:4 * (g + 1) * P, :].rearrange(
                            "(ht p) o -> p ht o", p=P),
                    )
                return t1, t2

            ios["s"]["w1"], ios["s"]["w2"] = load_wffn("s")

            # ---- B(f) | C(s) ----
            st(4)
            av(3)
            _emit_wo_residual(tc, pools, "s", ios["s"], (0, 1))
            st(5)
            av(4)
            _emit_wo_residual(tc, pools, "s", ios["s"], (2, 3))
            st(6)
            av(5)
            _emit_ln1(tc, pools, "s", ios["s"])
            st(7)
            av(6)
            av(7)
            _emit_wo_residual(tc, pools, "f", ios["f"], (0, 1))
            _emit_wo_residual(tc, pools, "f", ios["f"], (2, 3))
            # LN1(f) rsqrt still lands in the Exp table phase, before gelus.
            _emit_ln1(tc, pools, "f", ios["f"])

            ios["f"]["w1"], ios["f"]["w2"] = load_wffn("f")

            # ---- gelu phase: all 32 gelus back-to-back on ACT ----
            _emit_ffn1(tc, pools, "s", ios["s"], range(0, 16))
            _emit_ffn1(tc, pools, "f", ios["f"], range(0, 16))

            # ---- FFN2 + LN2 (rsqrts cluster after the gelus) ----
            _emit_ffn2(tc, pools, "s", ios["s"],
                       [(ct, ch) for ch in range(NCH) for ct in range(CT)])
            _emit_ln2(tc, pools, "s", ios["s"])
            _emit_ffn2(tc, pools, "f", ios["f"],
                       [(ct, ch) for ch in range(NCH) for ct in range(CT)])
            _emit_ln2(tc, pools, "f", ios["f"])

    nc.compile()
    return nc


# --------------------------------------------------------------------------
# host side
# --------------------------------------------------------------------------

_BF = ml_dtypes.bfloat16
_F8 = ml_dtypes.float8_e4m3
WS = 16.0  # host weight scale for fp8 matmuls (Wo, W1, W2)


def _head_perm():
    """Permuted output-channel order for Q/K projections.

    Tile t = 2g+hi, partition 32*h4+lo  ->  orig channel (4g+h4)*64+32*hi+lo.
    """
    perm = np.zeros(C, dtype=np.int64)
    i = 0
    for g in range(2):
        for hi in range(2):
            for h4 in range(4):
                for lo in range(32):
                    perm[i] = (4 * g + h4) * 64 + 32 * hi + lo
                    i += 1
    return perm


def _prep_shared_inputs(inputs):
    """Host-side weight prep: transposes, casts, permutations, 16x scales."""
    sh = {}
    perm = _head_perm()
    for p, ap in (("s", "s_"), ("f", "f_")):
        wq, wk, wv, wo = (inputs[ap + n] for n in ("Wq", "Wk", "Wv", "Wo"))
        # wqt columns = permuted W rows (head-grouped d-split layout)
        sh[f"{p}_wqt"] = np.ascontiguousarray(wq.T[:, perm]).astype(_BF)
        sh[f"{p}_wkt"] = np.ascontiguousarray(wk.T[:, perm]).astype(_BF)
        sh[f"{p}_wvt"] = np.ascontiguousarray(wv.T).astype(_BF)
        sh[f"{p}_wot"] = np.ascontiguousarray(wo.T * WS).astype(_F8)
        w1 = inputs[f"{p}ffn_W1"]
        w2 = inputs[f"{p}ffn_W2"]
        sh[f"{p}_w1t"] = np.ascontiguousarray(w1.T * WS).astype(_F8)
        sh[f"{p}_w2t"] = np.ascontiguousarray(w2.T * WS).astype(_F8)
        sh[f"{p}_bo16"] = np.ascontiguousarray(
            (inputs[ap + "bo"] * WS).reshape(CT, P).T
        ).astype(np.float32)
        for nm, key, scale in (
            ("n1w", f"{p}n1_w", 1.0), ("n1b", f"{p}n1_b", 1.0),
            ("n1w16", f"{p}n1_w", WS), ("n1b16", f"{p}n1_b", WS),
            ("n2w", f"{p}n2_w", 1.0), ("n2b", f"{p}n2_b", 1.0),
        ):
            sh[f"{p}_{nm}"] = np.ascontiguousarray(
                (inputs[key] * scale).reshape(CT, P).T
            ).astype(np.float32)
        sh[f"{p}_b1"] = np.ascontiguousarray(
            inputs[f"{p}ffn_b1"].reshape(HT, P).T
        ).astype(np.float32)
        sh[f"{p}_b216"] = np.ascontiguousarray(
            (inputs[f"{p}ffn_b2"] * WS).reshape(CT, P).T
        ).astype(np.float32)
    return sh


def _rename_ln(inputs):
    out = dict(inputs)
    for p in "sf":
        for i in "12":
            for wb in "wb":
                out[f"{p}n{i}_{wb}"] = inputs[f"{p}n{i}_{wb}"]
    return out


def make_in_maps(inputs):
    inputs = _rename_ln(inputs)
    shared = _prep_shared_inputs(inputs)
    xs = np.ascontiguousarray(inputs["spatial_feat"].reshape(B, C, HW))
    xf = np.ascontiguousarray(inputs["freq_feat"].reshape(B, C, HW))
    in_maps = []
    for b in range(N_CORES):
        m = dict(shared)
        m["x_s16"] = (xs[b] * WS).astype(_BF)
        m["x_f16"] = (xf[b] * WS).astype(_BF)
        m["x_sbf"] = xs[b].astype(_BF)
        m["x_fbf"] = xf[b].astype(_BF)
        in_maps.append(m)
    return in_maps


_CACHED = {}


def _get_program():
    if "nc" not in _CACHED:
        _CACHED["nc"] = build_program()
    return _CACHED["nc"]


def run_on_hw(inputs, trace=False, trace_kwargs=None):
    from concourse.bass_utils import run_bass_kernel_spmd

    nc = _get_program()
    in_maps = make_in_maps(inputs)
    res = run_bass_kernel_spmd(
        nc, in_maps, list(range(N_CORES)), trace=trace,
        **(dict(trace_kwargs=trace_kwargs) if trace_kwargs else {}),
    )
    s = np.stack([res.results[b]["out_s"] for b in range(B)])
    f = np.stack([res.results[b]["out_f"] for b in range(B)])
    s = s.reshape(B, C, H_IMG, W_IMG).astype(np.float32)
    f = f.reshape(B, C, H_IMG, W_IMG).astype(np.float32)
    return (s, f), res


def kernel(**inputs):
    out, _ = run_on_hw(inputs, trace=False)
    return out
